# revision 22
# baseline (speedup 1.0000x reference)
"""Trainium2 Bass kernel for nn_Attention_61795989455034 (sparse_attention).

reference computes, per batch b:
  tgt_p   = tgt[b] @ W_lin^T                       (1, 512)
  align   = tgt_p @ src[b]^T                       (1, 2048)
  mask_   = mask | onehot(prev_idxs[b])            (1, 2048) bool
  logits  = softmax(where(mask_, -inf, align))     (1, 2048)
  attn_h  = W_conv @ src[b]^T + b_conv             (512, 2048)
returns (attn_h, logits, mask_).

Strategy: data-parallel over batch across 8 NeuronCores (8 batches/core).
Per core the dominant work is the 1x1 conv (8 x [512x512x2048] matmul,
~8.6 GFLOP) plus 32 MB in / 32 MB out of DMA -- a compute/memory "ridge".
src arrives s-major; the TensorEngine contracts over partitions, so src is
transposed on-chip via PE transpose-mode into [d, s] tiles which feed both
the conv matmul and the align matmul (fp32r = TF32-rate, fp32 accumulate).
The align row for batch b is accumulated into row b of a shared [8, 512]
PSUM tile by using a stationary operand that is zero except in column b.
Softmax (mask add, -max, exp, 1/sum) runs on VectorE/ScalarE. mask_ is
produced on host (pure index bookkeeping).
"""

import numpy as np

import concourse.bass as bass
import concourse.bacc as bacc
import concourse.tile as tile
from concourse import mybir
from concourse.bass_utils import run_bass_kernel_spmd
from concourse.masks import make_identity
from contextlib import ExitStack

BZ, S, D = 64, 2048, 512
NCORES = 8
BL = BZ // NCORES          # batches per core
NPAN = S // 512            # s-panels per batch
F32 = mybir.dt.float32
F32R = mybir.dt.float32r

NEG_BIG = np.float32(-1e30)


def _build() -> bass.Bass:
    nc = bacc.Bacc(trn_type="TRN2")
    src_d = nc.declare_dram_parameter("src", [BL, S, D], F32, isOutput=False)
    maskadd_d = nc.declare_dram_parameter("maskadd", [BL, S], F32, isOutput=False)
    # packed [wlT | wcT | tgtT] along columns so PE consumers depend on ONE DMA
    wghts_d = nc.declare_dram_parameter("wghts", [D, 2 * D + BL], F32R, isOutput=False)
    bconv_d = nc.declare_dram_parameter("bconv", [128, 4], F32, isOutput=False)
    attn_d = nc.declare_dram_parameter("attn", [BL, D, S], F32, isOutput=True)
    logits_d = nc.declare_dram_parameter("logits", [BL, S], F32, isOutput=True)

    with tile.TileContext(nc) as tc:
        with ExitStack() as ctx:
            const = ctx.enter_context(tc.tile_pool(name="const", bufs=1))
            natp = ctx.enter_context(tc.tile_pool(name="nat", bufs=6))
            srcTp = ctx.enter_context(tc.tile_pool(name="srcT", bufs=4))
            outp = ctx.enter_context(tc.tile_pool(name="outc", bufs=8))
            alnp = ctx.enter_context(tc.tile_pool(name="aln", bufs=1))
            tps = ctx.enter_context(tc.tile_pool(name="tps", bufs=3, space="PSUM"))
            cps = ctx.enter_context(tc.tile_pool(name="cps", bufs=3, space="PSUM"))
            aps = ctx.enter_context(tc.tile_pool(name="aps", bufs=2, space="PSUM"))

            ident = const.tile([128, 128], F32)
            make_identity(nc, ident)

            # prefetch the first src panel before the (larger) weight load so
            # the PE can start transposing as early as possible
            nat0 = natp.tile([128, 4, 512], F32, tag="nat", name="nat0")
            nc.sync.dma_start(
                out=nat0,
                in_=src_d[0, 0:512, :].rearrange("(i q) d -> q i d", q=128),
            )

            # transpose the first panel immediately (PE work that needs no
            # weights) so the PE pipeline starts ~10us earlier
            srcT0 = srcTp.tile([128, 4, 512], F32R, tag="srcT", name="srcT0")
            for j in range(4):
                tp = tps.tile([128, 512], F32, tag="tps", name=f"tp0{j}")
                for i in range(4):
                    nc.tensor.transpose(
                        tp[:, i * 128:(i + 1) * 128],
                        nat0[:, i, j * 128:(j + 1) * 128],
                        ident,
                    )
                nc.scalar.copy(out=srcT0[:, j, :], in_=tp)

            # weights: [d_in(part), j, col] with d_in = j*128 + p
            wg_sb = const.tile([128, 4, 2 * D + BL], F32R)
            nc.sync.dma_start(
                out=wg_sb, in_=wghts_d[:, :].rearrange("(j p) e -> p j e", p=128)
            )
            bc_sb = const.tile([128, 4], F32)
            nc.sync.dma_start(out=bc_sb, in_=bconv_d[:, :])

            # tgt_pT[d, b] as 8 per-batch stationary tiles [d_in(part), j, col],
            # zero except column b (one shared PSUM tile row-accumulates all
            # batches' align rows).
            masked = [
                const.tile([128, 4, BL], F32R, tag=f"msk{b}", name=f"msk{b}")
                for b in range(BL)
            ]
            stage = [
                const.tile([128, 4, BL], F32, tag=f"stg{b}", name=f"stg{b}")
                for b in range(BL)
            ]
            for b in range(BL):
                nc.gpsimd.memset(stage[b], 0.0)
            for m in range(4):
                tp = tps.tile([128, BL], F32, tag="tps")
                for j in range(4):
                    nc.tensor.matmul(
                        tp,
                        wg_sb[:, j, m * 128:(m + 1) * 128],
                        wg_sb[:, j, 2 * D:2 * D + BL],
                        start=(j == 0),
                        stop=(j == 3),
                    )
                for b in range(BL):
                    nc.vector.tensor_copy(
                        out=stage[b][:, m, b:b + 1], in_=tp[:, b:b + 1]
                    )
            for b in range(BL):
                nc.vector.tensor_copy(out=masked[b], in_=stage[b])

            mk_sb = const.tile([BL, S], F32)
            nc.sync.dma_start(out=mk_sb, in_=maskadd_d[:, :])

            align = alnp.tile([BL, S], F32)
            pmax = alnp.tile([BL, 4], F32)

            for p in range(NPAN):
                ap_ps = aps.tile([BL, 512], F32, tag="aps")
                for b in range(BL):
                    if p == 0 and b == 0:
                        srcT = srcT0
                    else:
                        nat = natp.tile([128, 4, 512], F32, tag="nat")
                        nc.sync.dma_start(
                            out=nat,
                            in_=src_d[b, p * 512:(p + 1) * 512, :].rearrange(
                                "(i q) d -> q i d", q=128
                            ),
                        )
                        srcT = srcTp.tile([128, 4, 512], F32R, tag="srcT")
                        for j in range(4):
                            tp = tps.tile([128, 512], F32, tag="tps")
                            for i in range(4):
                                nc.tensor.transpose(
                                    tp[:, i * 128:(i + 1) * 128],
                                    nat[:, i, j * 128:(j + 1) * 128],
                                    ident,
                                )
                            nc.scalar.copy(out=srcT[:, j, :], in_=tp)
                    for j in range(4):
                        nc.tensor.matmul(
                            ap_ps,
                            masked[b][:, j, :],
                            srcT[:, j, :],
                            start=(b == 0 and j == 0),
                            stop=(b == BL - 1 and j == 3),
                        )
                    for m in range(4):
                        cp = cps.tile([128, 512], F32, tag="cps")
                        for j in range(4):
                            nc.tensor.matmul(
                                cp,
                                wg_sb[:, j, D + m * 128:D + (m + 1) * 128],
                                srcT[:, j, :],
                                start=(j == 0),
                                stop=(j == 3),
                            )
                        oc = outp.tile([128, 512], F32, tag="oc")
                        if m % 2 == 0:
                            nc.vector.tensor_scalar_add(
                                out=oc, in0=cp, scalar1=bc_sb[:, m:m + 1]
                            )
                        else:
                            nc.scalar.activation(
                                out=oc,
                                in_=cp,
                                func=mybir.ActivationFunctionType.Identity,
                                bias=bc_sb[:, m:m + 1],
                                scale=1.0,
                            )
                        nc.gpsimd.dma_start(
                            out=attn_d[b, m * 128:(m + 1) * 128, p * 512:(p + 1) * 512],
                            in_=oc,
                        )
                nc.vector.tensor_tensor(
                    out=align[:, p * 512:(p + 1) * 512],
                    in0=ap_ps,
                    in1=mk_sb[:, p * 512:(p + 1) * 512],
                    op=mybir.AluOpType.add,
                )
                nc.vector.tensor_reduce(
                    out=pmax[:, p:p + 1],
                    in_=align[:, p * 512:(p + 1) * 512],
                    axis=mybir.AxisListType.X,
                    op=mybir.AluOpType.max,
                )

            # softmax over the full row (matches jax.nn.softmax numerics)
            negmx = alnp.tile([BL, 1], F32)
            nc.vector.tensor_reduce(
                out=negmx,
                in_=pmax,
                axis=mybir.AxisListType.X,
                op=mybir.AluOpType.max,
                negate=True,
            )
            probs = alnp.tile([BL, S], F32)
            sumexp = alnp.tile([BL, 1], F32)
            nc.scalar.activation(
                out=probs,
                in_=align,
                func=mybir.ActivationFunctionType.Exp,
                bias=negmx[:, :],
                scale=1.0,
                accum_out=sumexp[:, :],
            )
            rsum = alnp.tile([BL, 1], F32)
            nc.vector.reciprocal(out=rsum, in_=sumexp)
            nc.vector.tensor_scalar_mul(out=probs, in0=probs, scalar1=rsum[:, :])
            nc.sync.dma_start(out=logits_d[:, :], in_=probs)

    nc.finalize()
    return nc


_NC_CACHE: bass.Bass | None = None


def _get_nc() -> bass.Bass:
    global _NC_CACHE
    if _NC_CACHE is None:
        _NC_CACHE = _build()
    return _NC_CACHE


def _prep(src, tgt, mask, prev_idxs, W_lin, W_conv, b_conv):
    """Shard + host-side index prep. Returns (in_maps, mask_)."""
    src = np.ascontiguousarray(np.asarray(src), dtype=np.float32)
    tgt = np.asarray(tgt, dtype=np.float32)
    mask = np.asarray(mask)
    prev = np.asarray(prev_idxs)
    W_lin = np.asarray(W_lin, dtype=np.float32)
    W_conv = np.asarray(W_conv, dtype=np.float32)
    b_conv = np.asarray(b_conv, dtype=np.float32)

    mask_ = np.array(mask, dtype=bool, copy=True)
    mask_[np.arange(BZ), :, prev] = True
    maskadd = np.where(mask_[:, 0, :], NEG_BIG, np.float32(0.0)).astype(np.float32)

    wlT = np.ascontiguousarray(W_lin.T)
    wcT = np.ascontiguousarray(W_conv.T)
    bc = np.ascontiguousarray(b_conv.reshape(4, 128).T)

    in_maps = []
    for c in range(NCORES):
        sl = slice(c * BL, (c + 1) * BL)
        in_maps.append(
            {
                "src": src[sl],
                "maskadd": np.ascontiguousarray(maskadd[sl]),
                "wghts": np.concatenate(
                    [wlT, wcT, np.ascontiguousarray(tgt[sl, 0, :].T)], axis=1
                ),
                "bconv": bc,
            }
        )
    return in_maps, mask_


def kernel(src, tgt, mask, prev_idxs, W_lin, W_out, W_conv, b_conv):
    nc = _get_nc()
    in_maps, mask_ = _prep(src, tgt, mask, prev_idxs, W_lin, W_conv, b_conv)
    res = run_bass_kernel_spmd(nc, in_maps, core_ids=list(range(NCORES)))
    attn = np.concatenate([res.results[c]["attn"] for c in range(NCORES)], axis=0)
    logits = np.concatenate(
        [res.results[c]["logits"] for c in range(NCORES)], axis=0
    ).reshape(BZ, 1, S)
    return attn, logits, mask_


# revision 23
# speedup vs baseline: 1.0189x; 1.0189x over previous
"""Trainium2 Bass kernel for nn_Attention_61795989455034 (sparse_attention).

reference computes, per batch b:
  tgt_p   = tgt[b] @ W_lin^T                       (1, 512)
  align   = tgt_p @ src[b]^T                       (1, 2048)
  mask_   = mask | onehot(prev_idxs[b])            (1, 2048) bool
  logits  = softmax(where(mask_, -inf, align))     (1, 2048)
  attn_h  = W_conv @ src[b]^T + b_conv             (512, 2048)
returns (attn_h, logits, mask_).

Strategy: data-parallel over batch across 8 NeuronCores (8 batches/core).
Per core the dominant work is the 1x1 conv (8 x [512x512x2048] matmul,
~8.6 GFLOP) plus 32 MB in / 32 MB out of DMA -- a compute/memory "ridge".
src arrives s-major; the TensorEngine contracts over partitions, so src is
transposed on-chip via PE transpose-mode into [d, s] tiles which feed both
the conv matmul and the align matmul (fp32r = TF32-rate, fp32 accumulate).
The align row for batch b is accumulated into row b of a shared [8, 512]
PSUM tile by using a stationary operand that is zero except in column b.
Softmax (mask add, -max, exp, 1/sum) runs on VectorE/ScalarE. mask_ is
produced on host (pure index bookkeeping).
"""

import numpy as np

import concourse.bass as bass
import concourse.bacc as bacc
import concourse.tile as tile
from concourse import mybir
from concourse.bass_utils import run_bass_kernel_spmd
from concourse.masks import make_identity
from contextlib import ExitStack

BZ, S, D = 64, 2048, 512
NCORES = 8
BL = BZ // NCORES          # batches per core
NPAN = S // 512            # s-panels per batch
F32 = mybir.dt.float32
F32R = mybir.dt.float32r

NEG_BIG = np.float32(-1e30)


def _build() -> bass.Bass:
    nc = bacc.Bacc(trn_type="TRN2")
    src_d = nc.declare_dram_parameter("src", [BL, S, D], F32, isOutput=False)
    maskadd_d = nc.declare_dram_parameter("maskadd", [BL, S], F32, isOutput=False)
    # packed [wlT | wcT | tgtT] along columns so PE consumers depend on ONE DMA
    wghts_d = nc.declare_dram_parameter("wghts", [D, 2 * D + BL], F32R, isOutput=False)
    bconv_d = nc.declare_dram_parameter("bconv", [128, 4], F32, isOutput=False)
    attn_d = nc.declare_dram_parameter("attn", [BL, D, S], F32, isOutput=True)
    logits_d = nc.declare_dram_parameter("logits", [BL, S], F32, isOutput=True)

    with tile.TileContext(nc) as tc:
        with ExitStack() as ctx:
            const = ctx.enter_context(tc.tile_pool(name="const", bufs=1))
            natp = ctx.enter_context(tc.tile_pool(name="nat", bufs=6))
            srcTp = ctx.enter_context(tc.tile_pool(name="srcT", bufs=4))
            outp = ctx.enter_context(tc.tile_pool(name="outc", bufs=8))
            alnp = ctx.enter_context(tc.tile_pool(name="aln", bufs=1))
            tps = ctx.enter_context(tc.tile_pool(name="tps", bufs=3, space="PSUM"))
            cps = ctx.enter_context(tc.tile_pool(name="cps", bufs=3, space="PSUM"))
            aps = ctx.enter_context(tc.tile_pool(name="aps", bufs=2, space="PSUM"))

            ident = const.tile([128, 128], F32)
            make_identity(nc, ident)

            # prefetch the first src panel before the (larger) weight load so
            # the PE can start transposing as early as possible
            nat0 = natp.tile([128, 4, 512], F32, tag="nat", name="nat0")
            nc.sync.dma_start(
                out=nat0,
                in_=src_d[0, 0:512, :].rearrange("(i q) d -> q i d", q=128),
            )

            # transpose the first panel immediately (PE work that needs no
            # weights) so the PE pipeline starts ~10us earlier
            srcT0 = srcTp.tile([128, 4, 512], F32R, tag="srcT", name="srcT0")
            for j in range(4):
                tp = tps.tile([128, 512], F32, tag="tps", name=f"tp0{j}")
                for i in range(4):
                    nc.tensor.transpose(
                        tp[:, i * 128:(i + 1) * 128],
                        nat0[:, i, j * 128:(j + 1) * 128],
                        ident,
                    )
                nc.scalar.copy(out=srcT0[:, j, :], in_=tp)

            # weights: [d_in(part), j, col] with d_in = j*128 + p
            wg_sb = const.tile([128, 4, 2 * D + BL], F32R)
            nc.sync.dma_start(
                out=wg_sb, in_=wghts_d[:, :].rearrange("(j p) e -> p j e", p=128)
            )
            bc_sb = const.tile([128, 4], F32)
            nc.sync.dma_start(out=bc_sb, in_=bconv_d[:, :])

            # tgt_pT[d, b] as 8 per-batch stationary tiles [d_in(part), j, col],
            # zero except column b (one shared PSUM tile row-accumulates all
            # batches' align rows).
            masked = [
                const.tile([128, 4, BL], F32R, tag=f"msk{b}", name=f"msk{b}")
                for b in range(BL)
            ]
            stage = [
                const.tile([128, 4, BL], F32, tag=f"stg{b}", name=f"stg{b}")
                for b in range(BL)
            ]
            for b in range(BL):
                nc.gpsimd.memset(stage[b], 0.0)
            for m in range(4):
                tp = tps.tile([128, BL], F32, tag="tps")
                for j in range(4):
                    nc.tensor.matmul(
                        tp,
                        wg_sb[:, j, m * 128:(m + 1) * 128],
                        wg_sb[:, j, 2 * D:2 * D + BL],
                        start=(j == 0),
                        stop=(j == 3),
                    )
                for b in range(BL):
                    nc.vector.tensor_copy(
                        out=stage[b][:, m, b:b + 1], in_=tp[:, b:b + 1]
                    )
            for b in range(BL):
                nc.vector.tensor_copy(out=masked[b], in_=stage[b])

            mk_sb = const.tile([BL, S], F32)
            nc.sync.dma_start(out=mk_sb, in_=maskadd_d[:, :])

            align = alnp.tile([BL, S], F32)
            pmax = alnp.tile([BL, 4], F32)

            for p in range(NPAN):
                ap_ps = aps.tile([BL, 512], F32, tag="aps")
                for b in range(BL):
                    if p == 0 and b == 0:
                        srcT = srcT0
                    else:
                        nat = natp.tile([128, 4, 512], F32, tag="nat")
                        nc.sync.dma_start(
                            out=nat,
                            in_=src_d[b, p * 512:(p + 1) * 512, :].rearrange(
                                "(i q) d -> q i d", q=128
                            ),
                        )
                        srcT = srcTp.tile([128, 4, 512], F32R, tag="srcT")
                        for j in range(4):
                            tp = tps.tile([128, 512], F32, tag="tps")
                            for i in range(4):
                                nc.tensor.transpose(
                                    tp[:, i * 128:(i + 1) * 128],
                                    nat[:, i, j * 128:(j + 1) * 128],
                                    ident,
                                )
                            nc.scalar.copy(out=srcT[:, j, :], in_=tp)
                    for m in range(4):
                        cp = cps.tile([128, 512], F32, tag="cps")
                        for j in range(4):
                            nc.tensor.matmul(
                                cp,
                                wg_sb[:, j, D + m * 128:D + (m + 1) * 128],
                                srcT[:, j, :],
                                start=(j == 0),
                                stop=(j == 3),
                            )
                        oc = outp.tile([128, 512], F32, tag="oc")
                        if m % 2 == 0:
                            nc.vector.tensor_scalar_add(
                                out=oc, in0=cp, scalar1=bc_sb[:, m:m + 1]
                            )
                        else:
                            nc.scalar.activation(
                                out=oc,
                                in_=cp,
                                func=mybir.ActivationFunctionType.Identity,
                                bias=bc_sb[:, m:m + 1],
                                scale=1.0,
                            )
                        nc.gpsimd.dma_start(
                            out=attn_d[b, m * 128:(m + 1) * 128, p * 512:(p + 1) * 512],
                            in_=oc,
                        )
                        if m == 3:
                            for j in range(4):
                                nc.tensor.matmul(
                                    ap_ps,
                                    masked[b][:, j, :],
                                    srcT[:, j, :],
                                    start=(b == 0 and j == 0),
                                    stop=(b == BL - 1 and j == 3),
                                )
                nc.vector.tensor_tensor(
                    out=align[:, p * 512:(p + 1) * 512],
                    in0=ap_ps,
                    in1=mk_sb[:, p * 512:(p + 1) * 512],
                    op=mybir.AluOpType.add,
                )
                nc.vector.tensor_reduce(
                    out=pmax[:, p:p + 1],
                    in_=align[:, p * 512:(p + 1) * 512],
                    axis=mybir.AxisListType.X,
                    op=mybir.AluOpType.max,
                )

            # softmax over the full row (matches jax.nn.softmax numerics)
            negmx = alnp.tile([BL, 1], F32)
            nc.vector.tensor_reduce(
                out=negmx,
                in_=pmax,
                axis=mybir.AxisListType.X,
                op=mybir.AluOpType.max,
                negate=True,
            )
            probs = alnp.tile([BL, S], F32)
            sumexp = alnp.tile([BL, 1], F32)
            nc.scalar.activation(
                out=probs,
                in_=align,
                func=mybir.ActivationFunctionType.Exp,
                bias=negmx[:, :],
                scale=1.0,
                accum_out=sumexp[:, :],
            )
            rsum = alnp.tile([BL, 1], F32)
            nc.vector.reciprocal(out=rsum, in_=sumexp)
            nc.vector.tensor_scalar_mul(out=probs, in0=probs, scalar1=rsum[:, :])
            nc.sync.dma_start(out=logits_d[:, :], in_=probs)

    nc.finalize()
    return nc


_NC_CACHE: bass.Bass | None = None


def _get_nc() -> bass.Bass:
    global _NC_CACHE
    if _NC_CACHE is None:
        _NC_CACHE = _build()
    return _NC_CACHE


def _prep(src, tgt, mask, prev_idxs, W_lin, W_conv, b_conv):
    """Shard + host-side index prep. Returns (in_maps, mask_)."""
    src = np.ascontiguousarray(np.asarray(src), dtype=np.float32)
    tgt = np.asarray(tgt, dtype=np.float32)
    mask = np.asarray(mask)
    prev = np.asarray(prev_idxs)
    W_lin = np.asarray(W_lin, dtype=np.float32)
    W_conv = np.asarray(W_conv, dtype=np.float32)
    b_conv = np.asarray(b_conv, dtype=np.float32)

    mask_ = np.array(mask, dtype=bool, copy=True)
    mask_[np.arange(BZ), :, prev] = True
    maskadd = np.where(mask_[:, 0, :], NEG_BIG, np.float32(0.0)).astype(np.float32)

    wlT = np.ascontiguousarray(W_lin.T)
    wcT = np.ascontiguousarray(W_conv.T)
    bc = np.ascontiguousarray(b_conv.reshape(4, 128).T)

    in_maps = []
    for c in range(NCORES):
        sl = slice(c * BL, (c + 1) * BL)
        in_maps.append(
            {
                "src": src[sl],
                "maskadd": np.ascontiguousarray(maskadd[sl]),
                "wghts": np.concatenate(
                    [wlT, wcT, np.ascontiguousarray(tgt[sl, 0, :].T)], axis=1
                ),
                "bconv": bc,
            }
        )
    return in_maps, mask_


def kernel(src, tgt, mask, prev_idxs, W_lin, W_out, W_conv, b_conv):
    nc = _get_nc()
    in_maps, mask_ = _prep(src, tgt, mask, prev_idxs, W_lin, W_conv, b_conv)
    res = run_bass_kernel_spmd(nc, in_maps, core_ids=list(range(NCORES)))
    attn = np.concatenate([res.results[c]["attn"] for c in range(NCORES)], axis=0)
    logits = np.concatenate(
        [res.results[c]["logits"] for c in range(NCORES)], axis=0
    ).reshape(BZ, 1, S)
    return attn, logits, mask_
